# revision 33
# baseline (speedup 1.0000x reference)
"""TRN2 Bass kernel for nn_AttentiveRouter (MoE routing).

Reference computation (per token, H=1024, F=2*H=2048, E=8, TOP_K=2):
    h  = gelu(x @ w1 + b1)            # exact erf gelu
    ew = h @ w2 + b2                  # expert logits [*, E]
    top-2 softmax -> dense masks, expert_usage, losses

Sharding: data-parallel over the batch dim (8 batch elements -> 8 cores),
router weights replicated. Per-core stats ([E] usage + nonzero counts) are
reduced on host to form the scalar losses.

Numerics: the top-2 selection must match a fp32 reference exactly (min gap
between 2nd/3rd expert logit ~2.8e-5), so both matmuls run as 3-pass fp16
hi/lo splits (a@b ~= ah@bh + al@bh + ah@bl) at full PE rate (~6e-7 rms
error in h), ordered k-major so passes sharing a stationary are adjacent
(HW-measured ~7ns/MM faster). HW-measured rates: fp16 [128,128]-stationary
N=512 matmuls sustain ~182ns incl. weight loads; a narrow [128,8] fp16
stationary costs ~255ns (no fast-weight-load below 128 columns) and fp32
costs ~434ns - hence matmul2 uses w2 with its 8 columns replicated 16x to
a full [128,128] stationary and streams h, producing ewT (all partition
groups equal, rows 0:8 read) which a PE transpose brings back to
token-major for the top-2 epilogue.
"""
import numpy as np
from contextlib import ExitStack

import concourse.mybir as mybir
import concourse.tile as tile
from concourse import bacc

F32 = mybir.dt.float32
F16 = mybir.dt.float16
AF = mybir.ActivationFunctionType
ALU = mybir.AluOpType
AX = mybir.AxisListType

H = 1024          # hidden
FF = 2048         # router MLP inner dim
E = 8             # experts
TOP_K = 2
B = 8             # batch (one element per core)
S = 2048          # tokens per core
N_CORES = 8
KC = H // 128     # 8 k-chunks for matmul1
FBN = FF // 128   # 16 f-blocks
TB = 512          # token block (psum free dim)
NB = S // TB      # 4 token blocks per core
K2 = FF // 128    # 16 k-chunks for matmul2

_CACHED_NC = None
_CACHED_EXEC = None


def _build():
    nc = bacc.Bacc("TRN2", target_bir_lowering=False, debug=False)

    # host-prepped layouts (hid-major, k-chunked on partitions)
    XH = nc.dram_tensor("XH", [128, KC, S], F16, kind="ExternalInput").ap()
    XL = nc.dram_tensor("XL", [128, KC, S], F16, kind="ExternalInput").ap()
    WH = nc.dram_tensor("WH", [128, KC, FF], F16, kind="ExternalInput").ap()
    WL = nc.dram_tensor("WL", [128, KC, FF], F16, kind="ExternalInput").ap()
    # w2 chunks with columns replicated 16x to a full [128,128] stationary:
    # FWL only engages at 128 weight columns, which HW-measures ~1.5x faster
    # per matmul than the narrow [128,8] stationary shape.
    W2H = nc.dram_tensor("W2H", [128, K2, 16 * E], F16, kind="ExternalInput").ap()
    W2L = nc.dram_tensor("W2L", [128, K2, 16 * E], F16, kind="ExternalInput").ap()
    B1T = nc.dram_tensor("B1T", [128, FBN], F32, kind="ExternalInput").ap()
    B2B = nc.dram_tensor("B2B", [128, E], F32, kind="ExternalInput").ap()
    IDT = nc.dram_tensor("IDT", [E, E], F32, kind="ExternalInput").ap()

    EW = nc.dram_tensor("EW", [S, E], F32, kind="ExternalOutput").ap()
    MASKS = nc.dram_tensor("MASKS", [S, E], F32, kind="ExternalOutput").ap()
    STATS = nc.dram_tensor("STATS", [E, 2], F32, kind="ExternalOutput").ap()

    with tile.TileContext(nc) as tc, ExitStack() as ctx:
        singles = ctx.enter_context(tc.tile_pool(name="singles", bufs=1))
        hpool = ctx.enter_context(tc.tile_pool(name="h", bufs=20))
        hfpool = ctx.enter_context(tc.tile_pool(name="hf", bufs=4))
        ewpool = ctx.enter_context(tc.tile_pool(name="ewp", bufs=4))
        mkpool = ctx.enter_context(tc.tile_pool(name="mkp", bufs=4))
        small = ctx.enter_context(tc.tile_pool(name="small", bufs=6))
        ps_h = ctx.enter_context(tc.tile_pool(name="ps_h", bufs=2, space="PSUM"))
        ps_ewt = ctx.enter_context(tc.tile_pool(name="ps_ewt", bufs=2, space="PSUM"))
        ps_tr = ctx.enter_context(tc.tile_pool(name="ps_tr", bufs=2, space="PSUM"))
        ps_st = ctx.enter_context(tc.tile_pool(name="ps_st", bufs=1, space="PSUM"))

        # ---- load weights / constants (consumption order) ----
        # w1 tiles per (k, f-group of 512) and x tiles per (k, blk): fine
        # DMA granularity so the first psum only waits on ~4MB, not 16MB.
        FG = 4  # f-groups (FBN//4 fb's each)
        wh_sb = {}
        wl_sb = {}
        for fg in range(FG):
            for k in range(KC):
                wh_sb[k, fg] = singles.tile([128, FF // FG], F16, tag=f"wh{k}_{fg}", name=f"wh{k}_{fg}")
                wl_sb[k, fg] = singles.tile([128, FF // FG], F16, tag=f"wl{k}_{fg}", name=f"wl{k}_{fg}")
        xh_sb = {}
        xl_sb = {}
        for blk in range(NB):
            for k in range(KC):
                xh_sb[k, blk] = singles.tile([128, TB], F16, tag=f"xh{k}_{blk}", name=f"xh{k}_{blk}")
                xl_sb[k, blk] = singles.tile([128, TB], F16, tag=f"xl{k}_{blk}", name=f"xl{k}_{blk}")

        w2h_sb = singles.tile([128, K2, 16 * E], F16, tag="w2h")
        w2l_sb = singles.tile([128, K2, 16 * E], F16, tag="w2l")
        b1_sb = singles.tile([128, FBN], F32, tag="b1")
        b2_sb = singles.tile([128, E], F32, tag="b2")
        idt_sb = singles.tile([E, E], F32, tag="idt")
        ones_sb = singles.tile([128, 1], F32, tag="ones")
        uacc = singles.tile([128, E], F32, tag="uacc")
        nacc = singles.tile([128, E], F32, tag="nacc")

        nc.sync.dma_start(w2h_sb[:], W2H[:])
        nc.sync.dma_start(w2l_sb[:], W2L[:])
        nc.sync.dma_start(b1_sb[:], B1T[:])
        nc.sync.dma_start(b2_sb[:], B2B[:])
        nc.sync.dma_start(idt_sb[:], IDT[:])
        nc.vector.memset(ones_sb[:], 1.0)
        nc.vector.memset(uacc[:], 0.0)
        nc.vector.memset(nacc[:], 0.0)

        # DMA order = first-psum consumption order: w f-group 0 + block-0 x
        # (~4MB) first, then the remaining w f-groups, then x blocks 1-3.
        FW = FF // FG
        for k in range(KC):
            nc.sync.dma_start(wh_sb[k, 0][:], WH[:, k, 0:FW])
            nc.sync.dma_start(xh_sb[k, 0][:], XH[:, k, 0:TB])
            nc.sync.dma_start(xl_sb[k, 0][:], XL[:, k, 0:TB])
            nc.sync.dma_start(wl_sb[k, 0][:], WL[:, k, 0:FW])
        for fg in range(1, FG):
            for k in range(KC):
                nc.sync.dma_start(wh_sb[k, fg][:], WH[:, k, fg * FW:(fg + 1) * FW])
                nc.sync.dma_start(wl_sb[k, fg][:], WL[:, k, fg * FW:(fg + 1) * FW])
        for blk in range(1, NB):
            for k in range(KC):
                nc.sync.dma_start(xh_sb[k, blk][:], XH[:, k, blk * TB:(blk + 1) * TB])
                nc.sync.dma_start(xl_sb[k, blk][:], XL[:, k, blk * TB:(blk + 1) * TB])

        # ---- main loop ----
        for blk in range(NB):
            hh_tiles = []
            hl_tiles = []
            for fb in range(FBN):
                p = ps_h.tile([128, TB], F32)
                fg, fo = fb // (FBN // FG), (fb % (FBN // FG)) * 128
                # k-major triplets: the first two share the wh[k] stationary,
                # which HW-measures ~7ns/MM faster than distinct stationaries.
                n_mm = 0
                for k in range(KC):
                    for wop, xop in ((wh_sb, xh_sb), (wh_sb, xl_sb), (wl_sb, xh_sb)):
                        nc.tensor.matmul(
                            p[:],
                            wop[k, fg][:, fo:fo + 128],
                            xop[k, blk][:],
                            start=(n_mm == 0),
                            stop=(n_mm == 3 * KC - 1),
                        )
                        n_mm += 1
                # gelu (exact erf table) then split h into fp16 hi/lo for mm2
                htf = hfpool.tile([128, TB], F32, tag="hf")
                nc.scalar.activation(htf[:], p[:], AF.Gelu, bias=b1_sb[:, fb:fb + 1])
                hh = hpool.tile([128, TB], F16, tag="hh")
                nc.vector.tensor_copy(hh[:], htf[:])
                hl = hpool.tile([128, TB], F16, tag="hl")
                nc.vector.tensor_sub(hl[:], htf[:], hh[:])
                hh_tiles.append(hh)
                hl_tiles.append(hl)

            # mm2: replicated w2 (fp16 hi/lo) stationary, h tiles moving.
            # out [128, TB] has the 8 logits replicated down the partitions;
            # rows 0:8 are read out.
            pewt = ps_ewt.tile([128, TB], F32)
            n_mm = 0
            for k2 in range(K2):
                for w2op, hop in ((w2h_sb, hh_tiles), (w2h_sb, hl_tiles), (w2l_sb, hh_tiles)):
                    nc.tensor.matmul(
                        pewt[:],
                        w2op[:, k2, :],
                        hop[k2][:],
                        start=(n_mm == 0),
                        stop=(n_mm == 3 * K2 - 1),
                    )
                    n_mm += 1
            ewt_sb = ewpool.tile([E, TB], F32, tag="ewt")
            nc.vector.tensor_copy(ewt_sb[:], pewt[0:E, :])

            for t in range(NB):
                g = blk * NB + t  # global 128-token tile index
                pe = ps_tr.tile([128, E], F32)
                nc.tensor.transpose(pe[:], ewt_sb[:, t * 128:(t + 1) * 128], idt_sb[:])
                # ---- top-2 softmax epilogue on [128 tokens, E] ----
                ew_sb = ewpool.tile([128, E], F32, tag="ew")
                nc.vector.tensor_add(ew_sb[:], pe[:], b2_sb[:])
                mx1 = small.tile([128, 1], F32, tag="mx1")
                nc.vector.tensor_reduce(mx1[:], ew_sb[:], axis=AX.X, op=ALU.max)
                tmp = ewpool.tile([128, E], F32, tag="tmp")
                nc.vector.tensor_scalar(
                    tmp[:], ew_sb[:], mx1[:], -1e30, op0=ALU.is_equal, op1=ALU.mult
                )
                ewm = ewpool.tile([128, E], F32, tag="ewm")
                nc.vector.tensor_add(ewm[:], ew_sb[:], tmp[:])
                mx2 = small.tile([128, 1], F32, tag="mx2")
                nc.vector.tensor_reduce(mx2[:], ewm[:], axis=AX.X, op=ALU.max)
                d = small.tile([128, 1], F32, tag="d")
                nc.vector.tensor_sub(d[:], mx2[:], mx1[:])
                a = small.tile([128, 1], F32, tag="a")
                nc.scalar.activation(a[:], d[:], AF.Exp)
                sden = small.tile([128, 1], F32, tag="s")
                nc.vector.tensor_scalar_add(sden[:], a[:], 1.0)
                r = small.tile([128, 1], F32, tag="r")
                nc.vector.reciprocal(r[:], sden[:])
                w2w = small.tile([128, 1], F32, tag="w2w")
                nc.vector.tensor_mul(w2w[:], a[:], r[:])
                t1 = ewpool.tile([128, E], F32, tag="t1")
                nc.vector.tensor_scalar(
                    t1[:], ew_sb[:], mx1[:], r[:], op0=ALU.is_equal, op1=ALU.mult
                )
                t2 = ewpool.tile([128, E], F32, tag="t2")
                nc.vector.tensor_scalar(
                    t2[:], ewm[:], mx2[:], w2w[:], op0=ALU.is_equal, op1=ALU.mult
                )
                mk = mkpool.tile([128, E], F32, tag="mk")
                nc.vector.tensor_add(mk[:], t1[:], t2[:])
                ind = ewpool.tile([128, E], F32, tag="ind")
                nc.vector.tensor_scalar(ind[:], mk[:], 0.0, None, op0=ALU.is_gt)
                nc.vector.tensor_add(uacc[:], uacc[:], mk[:])
                nc.vector.tensor_add(nacc[:], nacc[:], ind[:])
                nc.sync.dma_start(EW[g * 128:(g + 1) * 128, :], ew_sb[:])
                nc.sync.dma_start(MASKS[g * 128:(g + 1) * 128, :], mk[:])

        # ---- per-core stats: column sums over the 128 token lanes ----
        pu = ps_st.tile([E, 1], F32)
        nc.tensor.matmul(pu[:], uacc[:], ones_sb[:], start=True, stop=True)
        pn = ps_st.tile([E, 1], F32)
        nc.tensor.matmul(pn[:], nacc[:], ones_sb[:], start=True, stop=True)
        st = singles.tile([E, 2], F32, tag="st")
        nc.vector.tensor_copy(st[:, 0:1], pu[:])
        nc.vector.tensor_copy(st[:, 1:2], pn[:])
        nc.sync.dma_start(STATS[:], st[:])

    nc.compile()
    return nc


def _get_nc():
    global _CACHED_NC
    if _CACHED_NC is None:
        _CACHED_NC = _build()
    return _CACHED_NC


def _get_exec():
    """Build (once) a cached jitted SPMD executable for the Bass module.

    Mirrors concourse.bass2jax.run_bass_via_pjrt's multi-core path, but
    memoizes the jitted callable so repeated kernel() calls don't re-trace,
    re-compile, or re-upload anything beyond the inputs themselves.
    """
    global _CACHED_EXEC
    if _CACHED_EXEC is not None:
        return _CACHED_EXEC

    import jax
    from jax.experimental.shard_map import shard_map
    from jax.sharding import Mesh, PartitionSpec

    from concourse.bass2jax import (
        _bass_exec_p,
        install_neuronx_cc_hook,
        partition_id_tensor,
    )

    install_neuronx_cc_hook()
    nc = _get_nc()
    partition_name = nc.partition_id_tensor.name if nc.partition_id_tensor else None

    in_names, out_names, out_avals, zero_outs = [], [], [], []
    for alloc in nc.m.functions[0].allocations:
        if not isinstance(alloc, mybir.MemoryLocationSet):
            continue
        name = alloc.memorylocations[0].name
        if alloc.kind == "ExternalInput":
            if name != partition_name:
                in_names.append(name)
        elif alloc.kind == "ExternalOutput":
            shape = tuple(alloc.tensor_shape)
            dtype = mybir.dt.np(alloc.dtype)
            out_names.append(name)
            out_avals.append(jax.core.ShapedArray(shape, dtype))
            zero_outs.append(np.zeros(shape, dtype))
    n_params = len(in_names)
    all_names = in_names + out_names
    if partition_name is not None:
        all_names = all_names + [partition_name]
    donate = tuple(range(n_params, n_params + len(out_names)))

    def _body(*args):
        operands = list(args)
        if partition_name is not None:
            operands.append(partition_id_tensor())
        outs = _bass_exec_p.bind(
            *operands,
            out_avals=tuple(out_avals),
            in_names=tuple(all_names),
            out_names=tuple(out_names),
            lowering_input_output_aliases=(),
            sim_require_finite=True,
            sim_require_nnan=True,
            nc=nc,
        )
        return tuple(outs)

    devices = jax.devices()[:N_CORES]
    mesh = Mesh(np.asarray(devices), ("core",))
    nin = n_params + len(out_names)
    sharded = jax.jit(
        shard_map(
            _body,
            mesh=mesh,
            in_specs=(PartitionSpec("core"),) * nin,
            out_specs=(PartitionSpec("core"),) * len(out_names),
            check_rep=False,
        ),
        donate_argnums=donate,
        keep_unused=True,
    )
    _CACHED_EXEC = {
        "fn": sharded,
        "in_names": in_names,
        "out_names": out_names,
        "out_avals": out_avals,
        "zero_outs": zero_outs,
        "mesh": mesh,
    }
    return _CACHED_EXEC


def _run_spmd(in_maps):
    """Execute the kernel on N_CORES cores; returns list of per-core dicts."""
    ex = _get_exec()
    concat_in = [
        np.concatenate([in_maps[c][n] for c in range(N_CORES)], axis=0)
        for n in ex["in_names"]
    ]
    concat_zeros = [
        np.zeros((N_CORES * z.shape[0], *z.shape[1:]), z.dtype)
        for z in ex["zero_outs"]
    ]
    out_arrs = ex["fn"](*concat_in, *concat_zeros)
    return [
        {
            name: np.asarray(out_arrs[i]).reshape(N_CORES, *ex["out_avals"][i].shape)[c]
            for i, name in enumerate(ex["out_names"])
        }
        for c in range(N_CORES)
    ]


def _split16(a):
    hi = a.astype(np.float16)
    lo = (a - hi.astype(np.float32)).astype(np.float16)
    return hi, lo


def _prep_inputs(x, w1, b1, w2, b2):
    x = np.ascontiguousarray(np.asarray(x, dtype=np.float32))     # [B, S, H]
    w1 = np.asarray(w1, dtype=np.float32)                         # [H, FF]
    b1 = np.asarray(b1, dtype=np.float32)                         # [FF]
    w2 = np.asarray(w2, dtype=np.float32)                         # [FF, E]
    b2 = np.asarray(b2, dtype=np.float32)                         # [E]

    # shared (replicated) weight prep
    wt = w1.reshape(KC, 128, FF).transpose(1, 0, 2)               # [128, KC, FF]
    wh, wl = _split16(np.ascontiguousarray(wt))
    w2t = np.ascontiguousarray(w2.reshape(K2, 128, E).transpose(1, 0, 2))
    w2t = np.ascontiguousarray(np.tile(w2t, (1, 1, 16)))          # [128, K2, 128]
    w2h, w2lo = _split16(w2t)
    b1t = np.ascontiguousarray(b1.reshape(FBN, 128).T)            # [128, FBN]
    b2b = np.ascontiguousarray(np.broadcast_to(b2, (128, E)))
    idt = np.eye(E, dtype=np.float32)

    in_maps = []
    for c in range(N_CORES):
        xt = x[c].T.reshape(KC, 128, S).transpose(1, 0, 2)        # [128, KC, S]
        xh, xl = _split16(np.ascontiguousarray(xt))
        in_maps.append({
            "XH": xh, "XL": xl, "WH": wh, "WL": wl,
            "W2H": w2h, "W2L": w2lo, "B1T": b1t, "B2B": b2b, "IDT": idt,
        })
    return in_maps


def kernel(x, w1, b1, w2, b2):
    in_maps = _prep_inputs(x, w1, b1, w2, b2)
    results = _run_spmd(in_maps)

    ew = np.stack([r["EW"] for r in results]).astype(np.float32)     # [B, S, E]
    masks = np.stack([r["MASKS"] for r in results]).astype(np.float32)
    stats = np.stack([r["STATS"] for r in results])                   # [B, E, 2]

    usage = stats[:, :, 0].sum(axis=0, dtype=np.float32)
    expert_usage = (usage / usage.sum()).astype(np.float32)
    target = np.float32(1.0 / E)
    lb_loss = np.mean((expert_usage - target) ** 2, dtype=np.float32)
    nnz_total = stats[:, :, 1].sum(dtype=np.float32)
    sparsity = np.float32(nnz_total / (B * S) / TOP_K)
    total_loss = np.float32(lb_loss + np.float32(0.1) * sparsity)

    return ew, masks, total_loss, expert_usage
